# revision 10
# baseline (speedup 1.0000x reference)
"""Causal self-attention (S=2048, B=2, D=768, H=12) on 8 TRN2 NeuronCores.

Sharding: batch*heads across cores. Core c handles batch b = c//4 and the
3 heads hs = (c%4)*3 .. hs+2. Each core computes Q/K/V projections for its
heads, causal softmax(QK^T/sqrt(hd)) @ V, and its partial contribution to
the output projection y_part = att_cat @ wc_slice^T. The host gathers by
summing the 4 per-batch partials and adding the output bias.

Engine assignment (v3): Act runs ONLY the exp activations (it is the single
engine that can, and exp is the serial floor at ~66us); PSUM evacuation
(bias adds, V copies, AV normalize, attT/y copies) on DVE; causal-mask
multiplies on GpSimd (SBUF-only). Emission is software-pipelined kb-major:
head-0 projection first so exp starts ~3us in (inputs arrive via batched 3D
DMAs, xt in 4 column-chunks), head-1/2 + V projections fill PE gaps during
head-0 scores, AV runs one head behind its scores, and the per-query-block
tail (PE transpose -> yproj -> y DMA) trails two steps behind so
cross-engine latency hides. All DMA issue stays on the SP queue.

Numerics: matmul inputs in bf16, accumulation in fp32 PSUM, output partials
fp32. Scores are small (|s| < 3) so softmax skips the max-subtraction; the
denominator comes free from a ones-column appended to V.
"""

import numpy as np
import ml_dtypes

import concourse.bass as bass
import concourse.mybir as mybir
import concourse.tile as tile
from concourse import bacc
from concourse.bass_utils import run_bass_kernel_spmd

S = 2048  # sequence length
B = 2     # batch
D = 768   # model dim
H = 12    # heads
HD = 64   # head dim
NCORES = 8
HPC = 3   # heads per core
DC = HPC * HD          # 192: per-core head dims
VW = HPC * (HD + 1)    # 195: V columns incl per-head ones column
NQB = S // 128         # 16 query/key blocks
F32 = mybir.dt.float32
BF16 = mybir.dt.bfloat16
BF = ml_dtypes.bfloat16

TRACE = False          # set by test harness for profiled runs
LAST_RESULT = None     # BassKernelResults of the most recent run

_prog_cache = {}


def _score_chunks(kb):
    """Triangle-restricted (q0, n) chunks for key block kb, <=512 wide.

    Chunks start at kb*128 and stride 512 so that consecutive pairs pack
    contiguously into a [128, 1024] two-bank PSUM tile (each chunk within
    one bank) and one exp instruction can read the whole pair."""
    out = []
    q0 = kb * 128
    while q0 < S:
        out.append((q0, min(512, S - q0)))
        q0 += 512
    return out


def _build_program():
    nc = bacc.Bacc()

    xt = nc.declare_dram_parameter("xt", [D, S], BF16, isOutput=False)
    wqk = nc.declare_dram_parameter("wqk", [D, 2 * DC], BF16, isOutput=False)
    bqk = nc.declare_dram_parameter("bqk", [64, 6], F32, isOutput=False)
    wv = nc.declare_dram_parameter("wv", [D, VW], BF16, isOutput=False)
    g = nc.declare_dram_parameter("g", [DC, D], BF16, isOutput=False)
    y = nc.declare_dram_parameter("y", [S, D], BF16, isOutput=True)

    with tile.TileContext(nc) as tc:
        with (
            tc.tile_pool(name="const", bufs=1) as constp,
            tc.tile_pool(name="acts", bufs=1) as actsp,
            tc.tile_pool(name="pt", bufs=2) as ptp,
            tc.tile_pool(name="small", bufs=4) as smallp,
            tc.tile_pool(name="ys", bufs=4) as ysp,
            tc.tile_pool(name="psw", bufs=2, space="PSUM") as psw,
            tc.tile_pool(name="ps8", bufs=4, space="PSUM") as ps8,
        ):
            # ---- constants / weights ----
            # wqk + first xt quarter on SP so head-0 projection starts ASAP;
            # remaining loads issue from the (idle until exp) Act hwdge queue.
            ident = constp.tile([128, 128], BF16, tag="ident", name="ident")
            from concourse.masks import make_identity, make_upper_triangular
            make_identity(nc, ident[:])
            # mask[k, q] = 1 iff k <= q (upper triangular incl diagonal)
            mask = constp.tile([128, 128], BF16, tag="mask", name="mask")
            make_upper_triangular(nc, mask[:], val=1.0, diag=True)

            # wqk chunks packed side by side: col k*384+j <- wqk[k*128+p, j]
            wqk_sb = constp.tile([128, 6 * 2 * DC], BF16, tag="wqk", name="wqk")
            # xt in column-quarter DMAs: col k*2048+j <- xt[k*128+p, j]
            xt_sb = constp.tile([128, 6 * S], BF16, tag="xt", name="xt")

            def load_xt(n, eng, ks=slice(0, 6)):
                eng.dma_start(
                    xt_sb[:].rearrange("p (k j) -> p k j", k=6)[:, ks, n * 512:(n + 1) * 512],
                    xt[:, n * 512:(n + 1) * 512].rearrange("(k p) j -> p k j", k=6)[:, ks, :])

            # first xt quarter in two halves on SP (projection k-loop chases
            # them); wqk first on the Act hwdge queue.
            load_xt(0, nc.sync, slice(0, 3))
            nc.scalar.dma_start(
                wqk_sb[:].rearrange("p (k j) -> p k j", k=6),
                wqk[:, :].rearrange("(k p) j -> p k j", k=6))
            load_xt(0, nc.sync, slice(3, 6))
            # bias columns: [bq_h0, bk_h0, bq_h1, bk_h1, bq_h2, bk_h2]
            bqk_sb = constp.tile([64, 6], F32, tag="bqk", name="bqk")
            nc.scalar.dma_start(bqk_sb[:], bqk[:, :])
            load_xt(1, nc.sync)
            wv_sb = constp.tile([128, 6 * VW], BF16, tag="wv", name="wv")
            nc.scalar.dma_start(
                wv_sb[:].rearrange("p (k j) -> p k j", k=6),
                wv[:, :].rearrange("(k p) j -> p k j", k=6))
            load_xt(2, nc.sync)
            # g rows 0:128 at base partition 0; rows 128:192 parked at base
            # partition 64 so yproj's second matmul (lhsT = attT1[64:128])
            # sees both operands at the same base partition.
            g_sb0 = constp.tile([128, D], BF16, tag="g0", name="g0")
            nc.scalar.dma_start(g_sb0[:], g[0:128, :])
            g_sb1 = constp.tile([128, D], BF16, tag="g1", name="g1")
            nc.scalar.dma_start(g_sb1[64:128, :], g[128:192, :])
            load_xt(3, nc.sync)

            def xtc(k):
                return xt_sb[:, k * S:(k + 1) * S]

            def wqkc(k):
                return wqk_sb[:, k * 2 * DC:(k + 1) * 2 * DC]

            def wvc(k):
                return wv_sb[:, k * VW:(k + 1) * VW]

            # ---- activations ----
            # wqk column band h*128..h*128+128 holds [q_h | k_h] (64 each)
            qt = [actsp.tile([64, S], BF16, tag=f"qt{h}", name=f"qt{h}")
                  for h in range(HPC)]
            kt = [actsp.tile([64, S], BF16, tag=f"kt{h}", name=f"kt{h}")
                  for h in range(HPC)]
            v_sb = [actsp.tile([128, VW], BF16, tag=f"v{kb}", name=f"v{kb}")
                    for kb in range(NQB)]
            att3 = [actsp.tile([128, DC], BF16, tag=f"att{qi}", name=f"att{qi}")
                    for qi in range(NQB)]
            attT0 = actsp.tile([128, S], BF16, tag="attT0", name="attT0")
            # head-2 rows parked at partitions 64:128 to match g_sb1
            attT1 = actsp.tile([128, S], BF16, tag="attT1", name="attT1")

            # ---- emission helpers ----
            def emit_proj(h, ns=range(4)):
                """Q/K projection for head h -> qt[h], kt[h]."""
                for n in ns:
                    ps = ps8.tile([128, 512], F32, tag="ps", name="psqk")
                    for k in range(6):
                        nc.tensor.matmul(
                            ps[:], wqkc(k)[:, h * 128:(h + 1) * 128],
                            xtc(k)[:, n * 512:(n + 1) * 512],
                            start=(k == 0), stop=(k == 5))
                    nc.vector.tensor_scalar_add(
                        qt[h][:, n * 512:(n + 1) * 512], ps[0:64, :],
                        bqk_sb[:, 2 * h:2 * h + 1])
                    nc.vector.tensor_scalar_add(
                        kt[h][:, n * 512:(n + 1) * 512], ps[64:128, :],
                        bqk_sb[:, 2 * h + 1:2 * h + 2])

            def emit_vproj(kb):
                """V projection for key block kb -> v_sb[kb].

                No bias: +bv flows through the output projection as the
                constant bv @ wc.T, which the host folds into bc. The
                denominator ones-columns are memset separately."""
                ps = ps8.tile([128, 512], F32, tag="ps", name="psv")
                for k in range(6):
                    nc.tensor.matmul(
                        ps[:, 0:VW], xtc(k)[:, kb * 128:(kb + 1) * 128],
                        wvc(k)[:], start=(k == 0), stop=(k == 5))
                nc.vector.tensor_copy(v_sb[kb][:], ps[:, 0:VW])
                ones_ap = v_sb[kb][:].rearrange("p (h j) -> p h j", h=3)[:, :, 64:65]
                nc.gpsimd.memset(ones_ap, 1.0)

            pt = {}  # (h, kb) -> pt tile

            def emit_scores(h, kb):
                """Scores + exp + diagonal mask for (head h, key block kb).

                Score matmuls go in <=512 chunks (PSUM bank limit) but land
                pairwise in a [128, 1024] two-bank tile so a single exp
                covers both chunks (halves Act-engine instruction count)."""
                t = ptp.tile([128, S - kb * 128], BF16, tag=f"pt{kb}",
                             name=f"pt{kb}", bufs=(3 if kb < 6 else 2))
                pt[(h, kb)] = t
                chunks = _score_chunks(kb)
                for p in range(0, len(chunks), 2):
                    pair = chunks[p:p + 2]
                    ps = psw.tile([128, 1024], F32, tag="w", name="psw")
                    for i, (q0, n) in enumerate(pair):
                        nc.tensor.matmul(
                            ps[:, i * 512:i * 512 + n],
                            kt[h][:, kb * 128:(kb + 1) * 128],
                            qt[h][:, q0:q0 + n], start=True, stop=True)
                    tot = (512 + pair[1][1]) if len(pair) == 2 else pair[0][1]
                    o = pair[0][0] - kb * 128
                    nc.scalar.activation(
                        t[:, o:o + tot], ps[:, 0:tot],
                        mybir.ActivationFunctionType.Exp)
                # causal mask on the diagonal block: zero where k > q
                nc.gpsimd.tensor_mul(t[:, 0:128], t[:, 0:128], mask[:])

            def emit_av(h, qi):
                """AV for (head h, query block qi) -> normalized att3 cols."""
                po = ps8.tile([128, HD + 1], F32, tag="ps", name="po")
                for kb in range(qi + 1):
                    nc.tensor.matmul(
                        po[:], pt[(h, kb)][:, (qi - kb) * 128:(qi - kb + 1) * 128],
                        v_sb[kb][:, h * 65:h * 65 + 65],
                        start=(kb == 0), stop=(kb == qi))
                r = smallp.tile([128, 1], F32, tag="r", name="r")
                nc.vector.reciprocal(r[:], po[:, HD:HD + 1])
                nc.vector.tensor_scalar_mul(
                    att3[qi][:, h * 64:(h + 1) * 64], po[:, 0:HD], r[:])

            def emit_transp(qi):
                """Transpose att3[qi] on the PE, evacuate to attT0/attT1."""
                tr = ps8.tile([128, 256], BF16, tag="ps", name="tr")
                nc.tensor.transpose(tr[:, 0:128], att3[qi][:, 0:128], ident[:])
                nc.tensor.transpose(tr[0:64, 128:256], att3[qi][:, 128:192],
                                    ident[:])
                nc.vector.tensor_copy(attT0[:, qi * 128:(qi + 1) * 128],
                                      tr[:, 0:128])
                nc.vector.tensor_copy(attT1[64:128, qi * 128:(qi + 1) * 128],
                                      tr[0:64, 128:256])

            def emit_yproj(qi):
                """Output projection for query block qi and DMA out.

                For the last blocks the exp stream is finished, so half the
                PSUM evacuation goes to the then-idle Act engine."""
                ys = ysp.tile([128, D], BF16, tag="y", name="ys")
                for half in range(2):
                    ps = ps8.tile([128, 384], F32, tag="ps", name="psyp")
                    nc.tensor.matmul(
                        ps[:], attT0[:, qi * 128:(qi + 1) * 128],
                        g_sb0[:, half * 384:(half + 1) * 384],
                        start=True, stop=False)
                    nc.tensor.matmul(
                        ps[:], attT1[64:128, qi * 128:(qi + 1) * 128],
                        g_sb1[64:128, half * 384:(half + 1) * 384],
                        start=False, stop=True)
                    dst = ys[:, half * 384:(half + 1) * 384]
                    if qi >= NQB - 4 and half == 1:
                        nc.scalar.copy(dst, ps[:])
                    else:
                        nc.vector.tensor_copy(dst, ps[:])
                nc.sync.dma_start(y[qi * 128:(qi + 1) * 128, :], ys[:])

            # ---- phased, software-pipelined emission ----
            # Phase A: head-0 projection so exp can start early.
            emit_proj(0)

            # Phase B: head-0 scores; fill PE gaps with h1/h2 proj + V proj.
            for kb in range(NQB):
                emit_scores(0, kb)
                if kb < 2:
                    emit_proj(1, ns=(2 * kb, 2 * kb + 1))
                elif kb < 4:
                    emit_proj(2, ns=(2 * kb - 4, 2 * kb - 3))
                elif kb - 4 < 8:
                    emit_vproj(kb - 4)

            # Phase C: head-1 scores; head-0 AV; remaining V proj.
            for kb in range(NQB):
                emit_scores(1, kb)
                if kb % 2 == 0:
                    emit_vproj(8 + kb // 2)
                emit_av(0, kb)

            # Phase D: head-2 scores; head-1/2 AV lagged; per-qi tail
            # pipeline lagged two steps for cross-engine latency.
            for kb in range(NQB):
                emit_scores(2, kb)
                emit_av(1, kb)
                if kb >= 1:
                    emit_av(2, kb - 1)
                if kb >= 2:
                    emit_transp(kb - 2)
                if kb >= 3:
                    emit_yproj(kb - 3)
            emit_av(2, NQB - 1)
            for qi in range(NQB - 2, NQB):
                emit_transp(qi)
            for qi in range(NQB - 3, NQB):
                emit_yproj(qi)

    nc.finalize()
    return nc


def _prep_inputs(x, wq, bq, wk, bk, wv, bv, wc, bc):
    """Per-core input maps, all host-side slicing/transposition."""
    sc = 1.0 / np.sqrt(np.float32(HD))
    in_maps = []
    for c in range(NCORES):
        b = c // 4
        r0 = (c % 4) * HPC * HD
        xt = np.ascontiguousarray(x[:, b, :].T).astype(BF)
        # wqk columns: per-head bands [q_h | k_h] (64 each)
        wqk_cols = []
        bqk_cols = []
        for j in range(HPC):
            hr = slice(r0 + j * HD, r0 + (j + 1) * HD)
            wqk_cols.append(wq[hr] * sc)
            wqk_cols.append(wk[hr])
            bqk_cols.append(bq[hr] * sc)
            bqk_cols.append(bk[hr])
        wqk = np.ascontiguousarray(np.concatenate(wqk_cols, axis=0).T).astype(BF)
        bqk_t = np.stack(bqk_cols, axis=1).astype(np.float32)  # [64, 6]
        wva = np.zeros((D, VW), np.float32)
        for j in range(HPC):
            hr = slice(r0 + j * HD, r0 + (j + 1) * HD)
            wva[:D, j * 65:j * 65 + HD] = wv[hr].T
        rows = slice(r0, r0 + DC)
        g = np.ascontiguousarray(wc[:, rows].T).astype(BF)
        in_maps.append({
            "xt": xt,
            "wqk": wqk,
            "bqk": bqk_t,
            "wv": wva.astype(BF),
            "g": g,
        })
    return in_maps


def kernel(**inputs):
    global LAST_RESULT
    if "prog" not in _prog_cache:
        _prog_cache["prog"] = _build_program()
    nc = _prog_cache["prog"]

    args = {k: np.asarray(inputs[k], np.float32)
            for k in ("x", "wq", "bq", "wk", "bk", "wv", "bv", "wc", "bc")}
    in_maps = _prep_inputs(**args)
    res = run_bass_kernel_spmd(nc, in_maps, core_ids=list(range(NCORES)),
                               trace=TRACE)
    LAST_RESULT = res

    # V-bias contribution: att gets +bv per head dim, so y gets +bv @ wc.T
    bc_eff = args["bc"] + args["bv"] @ args["wc"].T
    out = np.empty((S, B, D), np.float32)
    for b in range(B):
        acc = res.results[4 * b]["y"].astype(np.float32)
        for c in range(4 * b + 1, 4 * b + 4):
            acc = acc + res.results[c]["y"]
        out[:, b, :] = acc + bc_eff[None, :]
    return out


# revision 11
# speedup vs baseline: 1.2199x; 1.2199x over previous
"""Causal self-attention (S=2048, B=2, D=768, H=12) on 8 TRN2 NeuronCores.

Sharding: batch*heads across cores. Core c handles batch b = c//4 and the
3 heads hs = (c%4)*3 .. hs+2. Each core computes Q/K/V projections for its
heads, causal softmax(QK^T/sqrt(hd)) @ V, and its partial contribution to
the output projection y_part = att_cat @ wc_slice^T. The host gathers by
summing the 4 per-batch partials and adding the output bias.

Engine assignment (v3): Act runs ONLY the exp activations (it is the single
engine that can, and exp is the serial floor at ~66us); PSUM evacuation
(bias adds, V copies, AV normalize, attT/y copies) on DVE; causal-mask
multiplies on GpSimd (SBUF-only). Emission is software-pipelined kb-major:
head-0 projection first so exp starts ~3us in (inputs arrive via batched 3D
DMAs, xt in 4 column-chunks), head-1/2 + V projections fill PE gaps during
head-0 scores, AV runs one head behind its scores, and the per-query-block
tail (PE transpose -> yproj -> y DMA) trails two steps behind so
cross-engine latency hides. All DMA issue stays on the SP queue.

Numerics: matmul inputs in bf16, accumulation in fp32 PSUM, output partials
fp32. Scores are small (|s| < 3) so softmax skips the max-subtraction; the
denominator comes free from a ones-column appended to V.
"""

import numpy as np
import ml_dtypes

import concourse.bass as bass
import concourse.mybir as mybir
import concourse.tile as tile
from concourse import bacc
from concourse.bass_utils import run_bass_kernel_spmd

S = 2048  # sequence length
B = 2     # batch
D = 768   # model dim
H = 12    # heads
HD = 64   # head dim
NCORES = 8
HPC = 3   # heads per core
DC = HPC * HD          # 192: per-core head dims
VW = HPC * (HD + 1)    # 195: V columns incl per-head ones column
NQB = S // 128         # 16 query/key blocks
F32 = mybir.dt.float32
BF16 = mybir.dt.bfloat16
BF = ml_dtypes.bfloat16

TRACE = False          # set by test harness for profiled runs
LAST_RESULT = None     # BassKernelResults of the most recent run

_prog_cache = {}


def _score_chunks(kb):
    """Triangle-restricted (q0, n) chunks for key block kb, <=512 wide.

    Chunks start at kb*128 and stride 512 so that consecutive pairs pack
    contiguously into a [128, 1024] two-bank PSUM tile (each chunk within
    one bank) and one exp instruction can read the whole pair."""
    out = []
    q0 = kb * 128
    while q0 < S:
        out.append((q0, min(512, S - q0)))
        q0 += 512
    return out


def _build_program():
    nc = bacc.Bacc()

    xt = nc.declare_dram_parameter("xt", [D, S], BF16, isOutput=False)
    wqk = nc.declare_dram_parameter("wqk", [D, 2 * DC], BF16, isOutput=False)
    bqk = nc.declare_dram_parameter("bqk", [64, 6], F32, isOutput=False)
    wv = nc.declare_dram_parameter("wv", [D, VW], BF16, isOutput=False)
    g = nc.declare_dram_parameter("g", [DC, D], BF16, isOutput=False)
    y = nc.declare_dram_parameter("y", [S, D], BF16, isOutput=True)

    with tile.TileContext(nc) as tc:
        with (
            tc.tile_pool(name="const", bufs=1) as constp,
            tc.tile_pool(name="acts", bufs=1) as actsp,
            tc.tile_pool(name="pt", bufs=2) as ptp,
            tc.tile_pool(name="small", bufs=4) as smallp,
            tc.tile_pool(name="ys", bufs=4) as ysp,
            tc.tile_pool(name="ps8", bufs=8, space="PSUM") as ps8,
        ):
            # ---- constants / weights ----
            # wqk + first xt quarter on SP so head-0 projection starts ASAP;
            # remaining loads issue from the (idle until exp) Act hwdge queue.
            ident = constp.tile([128, 128], BF16, tag="ident", name="ident")
            from concourse.masks import make_identity, make_upper_triangular
            make_identity(nc, ident[:])
            # mask[k, q] = 1 iff k <= q (upper triangular incl diagonal)
            mask = constp.tile([128, 128], BF16, tag="mask", name="mask")
            make_upper_triangular(nc, mask[:], val=1.0, diag=True)

            # wqk chunks packed side by side: col k*384+j <- wqk[k*128+p, j]
            wqk_sb = constp.tile([128, 6 * 2 * DC], BF16, tag="wqk", name="wqk")
            # xt in column-quarter DMAs: col k*2048+j <- xt[k*128+p, j]
            xt_sb = constp.tile([128, 6 * S], BF16, tag="xt", name="xt")

            def load_xt(n, eng, ks=slice(0, 6)):
                eng.dma_start(
                    xt_sb[:].rearrange("p (k j) -> p k j", k=6)[:, ks, n * 512:(n + 1) * 512],
                    xt[:, n * 512:(n + 1) * 512].rearrange("(k p) j -> p k j", k=6)[:, ks, :])

            # first xt quarter in two halves on SP (projection k-loop chases
            # them); wqk first on the Act hwdge queue.
            load_xt(0, nc.sync, slice(0, 3))
            nc.scalar.dma_start(
                wqk_sb[:].rearrange("p (k j) -> p k j", k=6),
                wqk[:, :].rearrange("(k p) j -> p k j", k=6))
            load_xt(0, nc.sync, slice(3, 6))
            # bias columns: [bq_h0, bk_h0, bq_h1, bk_h1, bq_h2, bk_h2]
            bqk_sb = constp.tile([64, 6], F32, tag="bqk", name="bqk")
            nc.scalar.dma_start(bqk_sb[:], bqk[:, :])
            load_xt(1, nc.sync)
            wv_sb = constp.tile([128, 6 * VW], BF16, tag="wv", name="wv")
            nc.scalar.dma_start(
                wv_sb[:].rearrange("p (k j) -> p k j", k=6),
                wv[:, :].rearrange("(k p) j -> p k j", k=6))
            load_xt(2, nc.sync)
            # g rows 0:128 at base partition 0; rows 128:192 parked at base
            # partition 64 so yproj's second matmul (lhsT = attT1[64:128])
            # sees both operands at the same base partition.
            g_sb0 = constp.tile([128, D], BF16, tag="g0", name="g0")
            nc.scalar.dma_start(g_sb0[:], g[0:128, :])
            g_sb1 = constp.tile([128, D], BF16, tag="g1", name="g1")
            nc.scalar.dma_start(g_sb1[64:128, :], g[128:192, :])
            load_xt(3, nc.sync)

            def xtc(k):
                return xt_sb[:, k * S:(k + 1) * S]

            def wqkc(k):
                return wqk_sb[:, k * 2 * DC:(k + 1) * 2 * DC]

            def wvc(k):
                return wv_sb[:, k * VW:(k + 1) * VW]

            # ---- activations ----
            # wqk column band h*128..h*128+128 holds [q_h | k_h] (64 each)
            qt = [actsp.tile([64, S], BF16, tag=f"qt{h}", name=f"qt{h}")
                  for h in range(HPC)]
            kt = [actsp.tile([64, S], BF16, tag=f"kt{h}", name=f"kt{h}")
                  for h in range(HPC)]
            v_sb = [actsp.tile([128, VW], BF16, tag=f"v{kb}", name=f"v{kb}")
                    for kb in range(NQB)]
            att3 = [actsp.tile([128, DC], BF16, tag=f"att{qi}", name=f"att{qi}")
                    for qi in range(NQB)]
            attT0 = actsp.tile([128, S], BF16, tag="attT0", name="attT0")
            # head-2 rows parked at partitions 64:128 to match g_sb1
            attT1 = actsp.tile([128, S], BF16, tag="attT1", name="attT1")

            # ---- emission helpers ----
            def emit_proj(h, ns=range(4)):
                """Q/K projection for head h -> qt[h], kt[h]."""
                for n in ns:
                    ps = ps8.tile([128, 512], F32, tag="ps", name="psqk")
                    for k in range(6):
                        nc.tensor.matmul(
                            ps[:], wqkc(k)[:, h * 128:(h + 1) * 128],
                            xtc(k)[:, n * 512:(n + 1) * 512],
                            start=(k == 0), stop=(k == 5))
                    nc.vector.tensor_scalar_add(
                        qt[h][:, n * 512:(n + 1) * 512], ps[0:64, :],
                        bqk_sb[:, 2 * h:2 * h + 1])
                    nc.vector.tensor_scalar_add(
                        kt[h][:, n * 512:(n + 1) * 512], ps[64:128, :],
                        bqk_sb[:, 2 * h + 1:2 * h + 2])

            def emit_vproj(kb):
                """V projection for key block kb -> v_sb[kb].

                No bias: +bv flows through the output projection as the
                constant bv @ wc.T, which the host folds into bc. The
                denominator ones-columns are memset separately."""
                ps = ps8.tile([128, 512], F32, tag="ps", name="psv")
                for k in range(6):
                    nc.tensor.matmul(
                        ps[:, 0:VW], xtc(k)[:, kb * 128:(kb + 1) * 128],
                        wvc(k)[:], start=(k == 0), stop=(k == 5))
                nc.vector.tensor_copy(v_sb[kb][:], ps[:, 0:VW])
                ones_ap = v_sb[kb][:].rearrange("p (h j) -> p h j", h=3)[:, :, 64:65]
                nc.gpsimd.memset(ones_ap, 1.0)

            pt = {}  # (h, kb) -> pt tile

            def emit_scores(h, kb):
                """Scores + exp + diagonal mask for (head h, key block kb).

                Score matmuls go in <=512 chunks (PSUM bank limit), each
                exp'd straight out of its bank."""
                t = ptp.tile([128, S - kb * 128], BF16, tag=f"pt{kb}",
                             name=f"pt{kb}", bufs=(3 if kb < 6 else 2))
                pt[(h, kb)] = t
                for (q0, n) in _score_chunks(kb):
                    ps = ps8.tile([128, 512], F32, tag="ps", name="psmm")
                    nc.tensor.matmul(
                        ps[:, :n], kt[h][:, kb * 128:(kb + 1) * 128],
                        qt[h][:, q0:q0 + n], start=True, stop=True)
                    nc.scalar.activation(
                        t[:, q0 - kb * 128:q0 - kb * 128 + n],
                        ps[:, :n], mybir.ActivationFunctionType.Exp)
                # causal mask on the diagonal block: zero where k > q
                nc.gpsimd.tensor_mul(t[:, 0:128], t[:, 0:128], mask[:])

            def emit_av(h, qi):
                """AV for (head h, query block qi) -> normalized att3 cols."""
                po = ps8.tile([128, HD + 1], F32, tag="ps", name="po")
                for kb in range(qi + 1):
                    nc.tensor.matmul(
                        po[:], pt[(h, kb)][:, (qi - kb) * 128:(qi - kb + 1) * 128],
                        v_sb[kb][:, h * 65:h * 65 + 65],
                        start=(kb == 0), stop=(kb == qi))
                r = smallp.tile([128, 1], F32, tag="r", name="r")
                nc.vector.reciprocal(r[:], po[:, HD:HD + 1])
                nc.vector.tensor_scalar_mul(
                    att3[qi][:, h * 64:(h + 1) * 64], po[:, 0:HD], r[:])

            def emit_transp(qi):
                """Transpose att3[qi] on the PE, evacuate to attT0/attT1."""
                tr = ps8.tile([128, 256], BF16, tag="ps", name="tr")
                nc.tensor.transpose(tr[:, 0:128], att3[qi][:, 0:128], ident[:])
                nc.tensor.transpose(tr[0:64, 128:256], att3[qi][:, 128:192],
                                    ident[:])
                nc.vector.tensor_copy(attT0[:, qi * 128:(qi + 1) * 128],
                                      tr[:, 0:128])
                nc.vector.tensor_copy(attT1[64:128, qi * 128:(qi + 1) * 128],
                                      tr[0:64, 128:256])

            def emit_yproj(qi):
                """Output projection for query block qi and DMA out.

                For the last blocks the exp stream is finished, so half the
                PSUM evacuation goes to the then-idle Act engine."""
                ys = ysp.tile([128, D], BF16, tag="y", name="ys")
                for half in range(2):
                    ps = ps8.tile([128, 384], F32, tag="ps", name="psyp")
                    nc.tensor.matmul(
                        ps[:], attT0[:, qi * 128:(qi + 1) * 128],
                        g_sb0[:, half * 384:(half + 1) * 384],
                        start=True, stop=False)
                    nc.tensor.matmul(
                        ps[:], attT1[64:128, qi * 128:(qi + 1) * 128],
                        g_sb1[64:128, half * 384:(half + 1) * 384],
                        start=False, stop=True)
                    dst = ys[:, half * 384:(half + 1) * 384]
                    if qi >= NQB - 4 and half == 1:
                        nc.scalar.copy(dst, ps[:])
                    else:
                        nc.vector.tensor_copy(dst, ps[:])
                nc.sync.dma_start(y[qi * 128:(qi + 1) * 128, :], ys[:])

            # ---- phased, software-pipelined emission ----
            # Phase A: head-0 projection so exp can start early.
            emit_proj(0)

            # Phase B: head-0 scores; fill PE gaps with h1/h2 proj + V proj.
            for kb in range(NQB):
                emit_scores(0, kb)
                if kb < 2:
                    emit_proj(1, ns=(2 * kb, 2 * kb + 1))
                elif kb < 4:
                    emit_proj(2, ns=(2 * kb - 4, 2 * kb - 3))
                elif kb - 4 < 8:
                    emit_vproj(kb - 4)

            # Phase C: head-1 scores; head-0 AV; remaining V proj.
            for kb in range(NQB):
                emit_scores(1, kb)
                if kb % 2 == 0:
                    emit_vproj(8 + kb // 2)
                emit_av(0, kb)

            # Phase D: head-2 scores; head-1/2 AV lagged; per-qi tail
            # pipeline lagged two steps for cross-engine latency.
            for kb in range(NQB):
                emit_scores(2, kb)
                emit_av(1, kb)
                if kb >= 1:
                    emit_av(2, kb - 1)
                if kb >= 2:
                    emit_transp(kb - 2)
                if kb >= 3:
                    emit_yproj(kb - 3)
            emit_av(2, NQB - 1)
            for qi in range(NQB - 2, NQB):
                emit_transp(qi)
            for qi in range(NQB - 3, NQB):
                emit_yproj(qi)

    nc.finalize()
    return nc


def _prep_inputs(x, wq, bq, wk, bk, wv, bv, wc, bc):
    """Per-core input maps, all host-side slicing/transposition."""
    sc = 1.0 / np.sqrt(np.float32(HD))
    in_maps = []
    for c in range(NCORES):
        b = c // 4
        r0 = (c % 4) * HPC * HD
        xt = np.ascontiguousarray(x[:, b, :].T).astype(BF)
        # wqk columns: per-head bands [q_h | k_h] (64 each)
        wqk_cols = []
        bqk_cols = []
        for j in range(HPC):
            hr = slice(r0 + j * HD, r0 + (j + 1) * HD)
            wqk_cols.append(wq[hr] * sc)
            wqk_cols.append(wk[hr])
            bqk_cols.append(bq[hr] * sc)
            bqk_cols.append(bk[hr])
        wqk = np.ascontiguousarray(np.concatenate(wqk_cols, axis=0).T).astype(BF)
        bqk_t = np.stack(bqk_cols, axis=1).astype(np.float32)  # [64, 6]
        wva = np.zeros((D, VW), np.float32)
        for j in range(HPC):
            hr = slice(r0 + j * HD, r0 + (j + 1) * HD)
            wva[:D, j * 65:j * 65 + HD] = wv[hr].T
        rows = slice(r0, r0 + DC)
        g = np.ascontiguousarray(wc[:, rows].T).astype(BF)
        in_maps.append({
            "xt": xt,
            "wqk": wqk,
            "bqk": bqk_t,
            "wv": wva.astype(BF),
            "g": g,
        })
    return in_maps


def kernel(**inputs):
    global LAST_RESULT
    if "prog" not in _prog_cache:
        _prog_cache["prog"] = _build_program()
    nc = _prog_cache["prog"]

    args = {k: np.asarray(inputs[k], np.float32)
            for k in ("x", "wq", "bq", "wk", "bk", "wv", "bv", "wc", "bc")}
    in_maps = _prep_inputs(**args)
    res = run_bass_kernel_spmd(nc, in_maps, core_ids=list(range(NCORES)),
                               trace=TRACE)
    LAST_RESULT = res

    # V-bias contribution: att gets +bv per head dim, so y gets +bv @ wc.T
    bc_eff = args["bc"] + args["bv"] @ args["wc"].T
    out = np.empty((S, B, D), np.float32)
    for b in range(B):
        acc = res.results[4 * b]["y"].astype(np.float32)
        for c in range(4 * b + 1, 4 * b + 4):
            acc = acc + res.results[c]["y"]
        out[:, b, :] = acc + bc_eff[None, :]
    return out
